# revision 30
# baseline (speedup 1.0000x reference)
"""Causal single-head attention block on 8 TRN2 NeuronCores.

Reference: Q=x@Wq, K=x@Wk, V=x@Wv; S=Q@K^T (no pre-softmax scaling);
causal mask; P=softmax(S); out=(P@V)/sqrt(64).
Shapes: x [4, 2048, 1024] f32, W* [1024, 64] f32 -> out [4, 2048, 64].

Sharding: 8 cores = 4 batches x 2 interleaved query-tile sets.
Core (b, jj) owns 8 query tiles of 128 rows:
  jj=0: g = {0,2,4,6,9,11,13,15},  jj=1: g = {1,3,5,7,8,10,12,14}
Both sets have equal causal work at 128-key granularity (sum g+1 = 68).

Host prep (per core): x[b] is transposed, cast to fp16, and its sixteen
128-row tiles are permuted into "slots": slots 0..7 = the core's own
query tiles ascending, slots 8..15 = the complementary tiles ascending.
This makes the device program identical across cores (SPMD) with all
per-core variation carried by DRAM data. Attention output is invariant
to key order; causality is handled by one shared triangular mask (the
diagonal tile always lands on a fixed block index) plus a per-core 0/1
scalar per query tile (the boundary tile is fully allowed or fully
forbidden) that is folded into a pre-scaled copy of the boundary V slot
("vz"), keeping masks off the critical path.

The x slots stream in ascending order (q-slot chunks first, then comp
chunks); S^T/exp/PV work is emitted per arrival, with PV accumulation
groups deferred one chunk past their exp dependencies so the in-order
PE never head-of-line blocks on a fresh exp, and the last two PV
groups split so only 1-2 matmuls remain gated on the final slots.

On-chip dataflow per core (x^T resident in SBUF, fp16):
  K^T|Q^T fused projection:  psum[kq, t] = [Wk|Wq]^T @ x^T   (1 cy/col)
  V natural direct:          psum[t, v]  = x^T-tile.T @ (Wv/8)
                             (1/sqrt(64) folded into Wv on host)
  S^T tile [t,q] = K^T-slot.T @ Q^T-tile   -> exp on ACT -> E^T bf16
  E^T diag block *= tri (shared, on GpSimd)
  out psum [q, 65] += E^T-block.T @ [V | 1]  (ones col gives row sums)
  out = psum[:, :64] * (1 / psum[:, 64])  (DVE reciprocal + scale)

A short stream of scratch matmuls warms the PE clock-ramp model while
the first DMAs land. fp16 for the Q/K path (S abs max ~60), bf16 for E
(exp(60) needs range) and V. Measured rel err ~5e-3.
"""

import sys

import numpy as np
import ml_dtypes

try:  # concourse ships in the TRN container; fall back to its known path
    import concourse  # noqa: F401
except ImportError:
    sys.path.insert(0, "/opt/trn_rl_repo")

B, T, C, DK = 4, 2048, 1024, 64
NLI = 8          # query tiles per core
NSLOT = 16       # key tiles (slots) per batch
NWARM = 13       # PE warmup matmuls (clock-ramp model: warm after ~3us)

_CACHE = {}


def _build():
    import concourse.bacc as bacc
    import concourse.tile as tile
    import concourse.mybir as mybir

    f32 = mybir.dt.float32
    f16 = mybir.dt.float16
    bf16 = mybir.dt.bfloat16
    EXP = mybir.ActivationFunctionType.Exp

    nc = bacc.Bacc("TRN2", target_bir_lowering=False, debug=False,
                   enable_asserts=False, num_devices=8)

    xt_d = nc.dram_tensor("xt", [128, NSLOT, 1024], f16, kind="ExternalInput").ap()
    wkq_d = nc.dram_tensor("wkq", [128, 8, 128], f16, kind="ExternalInput").ap()
    wv_d = nc.dram_tensor("wv", [128, 8, DK], f16, kind="ExternalInput").ap()
    msk_d = nc.dram_tensor("msk", [128, 136], bf16, kind="ExternalInput").ap()
    y_d = nc.dram_tensor("y", [128, NLI, DK], f32, kind="ExternalOutput").ap()

    with tile.TileContext(nc) as tc:
        with (
            tc.tile_pool(name="persist", bufs=1) as pp,
            tc.tile_pool(name="pmix", bufs=2, space="PSUM") as pmx,
            tc.tile_pool(name="pst", bufs=4, space="PSUM") as pst,
            tc.tile_pool(name="pout", bufs=2, space="PSUM") as pou,
        ):
            xt = pp.tile([128, NSLOT, 1024], f16, tag="xt", name="xt")
            kt = pp.tile([64, NSLOT, 128], f16, tag="kt", name="kt")
            qt = pp.tile([64, NLI, 128], f16, tag="qt", name="qt")
            vv = pp.tile([128, NSLOT, DK + 1], bf16, tag="vv", name="vv")
            vz = pp.tile([128, NLI, DK + 1], bf16, tag="vz", name="vz")
            wkq = pp.tile([128, 8, 128], f16, tag="wkq", name="wkq")
            wv = pp.tile([128, 8, DK], f16, tag="wv", name="wv")
            msk = pp.tile([128, 136], bf16, tag="msk", name="msk")
            tri = msk[:, 0:128]
            svec = pp.tile([128, NLI], f32, tag="svec", name="svec")
            yout = pp.tile([128, NLI, DK], f32, tag="yout", name="yout")
            rv = pp.tile([128, NLI], f32, tag="rv", name="rv")
            scr = pp.tile([128, 256], f16, tag="scr", name="scr")
            E = [pp.tile([128, (2 * li + 2) * 128], bf16, tag=f"E{li}",
                         name=f"E{li}") for li in range(NLI)]

            nc.vector.memset(scr, 0.0)
            nc.vector.memset(vv[:, :, DK:DK + 1], 1.0)

            # ---- DMA program: wkq, then single slots 0 and 1 (earliest
            # possible first projection), then slot pairs; output quarters
            # are issued after the x stream so they never delay it ----
            nc.sync.dma_start(wkq, wkq_d)
            nc.sync.dma_start(xt[:, 0, :], xt_d[:, 0, :])
            nc.sync.dma_start(xt[:, 1, :], xt_d[:, 1, :])
            nc.sync.dma_start(msk, msk_d)
            nc.vector.tensor_copy(svec, msk[:, 128:136])  # bf16 -> f32
            nc.sync.dma_start(wv, wv_d)
            for a, b in [(2, 4), (4, 6), (6, 8), (8, 10), (10, 12),
                         (12, 14)]:
                nc.sync.dma_start(xt[:, a:b, :], xt_d[:, a:b, :])
            nc.sync.dma_start(xt[:, 14, :], xt_d[:, 14, :])
            nc.sync.dma_start(xt[:, 15, :], xt_d[:, 15, :])
            nc.sync.dma_start(y_d[:, 0:2, :], yout[:, 0:2, :])
            nc.sync.dma_start(y_d[:, 2:4, :], yout[:, 2:4, :])
            nc.sync.dma_start(y_d[:, 4:6, :], yout[:, 4:6, :])

            # ---- PE warmup: keeps visit-time clock state warm while the
            # first x DMAs land ----
            for w in range(NWARM):
                pw = pmx.tile([128, 2, 128], f32, tag="pmix", name="pw")
                nc.tensor.matmul(pw[:, 0:2, :], scr[:, 0:128], scr,
                                 start=True, stop=True)

            def kq_proj(s0, nslots):
                """K^T|Q^T for slots [s0, s0+nslots); one psum tile+copy."""
                ps = pmx.tile([128, 2, 128], f32, tag="pmix", name="pkq")
                for i in range(nslots):
                    s = s0 + i
                    for ch in range(8):
                        nc.tensor.matmul(
                            ps[:, i, :],
                            wkq[:, ch, :],
                            xt[:, s, ch * 128:(ch + 1) * 128],
                            start=(ch == 0), stop=(ch == 7),
                        )
                nc.vector.tensor_copy(kt[:, s0:s0 + nslots, :],
                                      ps[0:64, 0:nslots, :])
                if s0 < NLI:  # Q only meaningful for the core's q-slots
                    nc.vector.tensor_copy(qt[:, s0:s0 + nslots, :],
                                          ps[64:128, 0:nslots, :])

            def v_proj(s0, nslots):
                """V natural (pre-scaled by 1/8) for slots [s0, s0+nslots)."""
                ps = pmx.tile([128, 2, 128], f32, tag="pmix", name="pv")
                for i in range(nslots):
                    s = s0 + i
                    for ch in range(8):
                        nc.tensor.matmul(
                            ps[:, i, 0:DK],
                            xt[:, s, ch * 128:(ch + 1) * 128],
                            wv[:, ch, :],
                            start=(ch == 0), stop=(ch == 7),
                        )
                nc.vector.tensor_copy(vv[:, s0:s0 + nslots, 0:DK],
                                      ps[:, 0:nslots, 0:DK])

            def s_blocks(li, blocks, tag_suffix=""):
                """S^T then exp for E[li] col blocks `blocks` (block j:
                key slot j if j<=li else 8+(j-li-1); q = slot li)."""
                nb = len(blocks)
                ps = pst.tile([128, 512], f32, tag="pst",
                              name=f"ps{li}{tag_suffix}")
                for i, j in enumerate(blocks):
                    s = j if j <= li else 8 + (j - li - 1)
                    nc.tensor.matmul(
                        ps[:, i * 128:(i + 1) * 128],
                        kt[:, s, :],
                        qt[:, li, :],
                        start=True, stop=True,
                    )
                j0 = blocks[0]
                nc.scalar.activation(
                    E[li][:, j0 * 128:(j0 + nb) * 128], ps[:, 0:nb * 128], EXP)
                if li in blocks:  # diagonal block: shared triangular mask
                    nc.gpsimd.tensor_mul(
                        E[li][:, li * 128:(li + 1) * 128],
                        E[li][:, li * 128:(li + 1) * 128], tri)

            def vz_make(li):
                # boundary V slot pre-multiplied by the per-core 0/1 scalar
                # (incl. the ones column -> masked keys add 0 to the rowsum)
                nc.gpsimd.tensor_scalar_mul(
                    vz[:, li, :], vv[:, 8 + li, :], svec[:, li:li + 1])

            po_t = {}

            def pv_mm(li, blocks, start, stop):
                if li not in po_t:
                    po_t[li] = pou.tile([128, DK + 1], f32, tag="pout",
                                        name=f"po{li}")
                po = po_t[li]
                last = blocks[-1]
                for j in blocks:
                    if j == 2 * li + 1:
                        rhs = vz[:, li, :]
                    else:
                        s = j if j <= li else 8 + (j - li - 1)
                        rhs = vv[:, s, :]
                    nc.tensor.matmul(
                        po, E[li][:, j * 128:(j + 1) * 128], rhs,
                        start=(start and j == blocks[0]),
                        stop=(stop and j == last),
                        skip_group_check=True,
                    )
                if stop:
                    nc.vector.reciprocal(rv[:, li:li + 1], po[:, DK:DK + 1])
                    nc.vector.tensor_scalar_mul(
                        yout[:, li, :], po[:, 0:DK], rv[:, li:li + 1])

            def pv(li):
                pv_mm(li, list(range(2 * li + 2)), True, True)

            # ---- main schedule (chunk-ordered: q-slot chunks 0..3 first,
            # then comp chunks; PV groups deferred one chunk past their exp
            # dependencies) ----
            # c0 (slots 0, 1; per-slot for earliest start)
            kq_proj(0, 1)
            kq_proj(1, 1)
            v_proj(0, 2)
            s_blocks(0, [0])
            s_blocks(1, [0, 1])
            # c1 (slots 2, 3)
            kq_proj(2, 2)
            v_proj(2, 2)
            s_blocks(2, [0, 1, 2])
            s_blocks(3, [0, 1, 2, 3])
            # c2 (slots 4, 5)
            kq_proj(4, 2)
            v_proj(4, 2)
            s_blocks(4, [0, 1, 2, 3], "a")
            s_blocks(4, [4], "b")
            s_blocks(5, [0, 1, 2, 3], "a")
            s_blocks(5, [4, 5], "b")
            # c3 (slots 6, 7)
            kq_proj(6, 2)
            v_proj(6, 2)
            s_blocks(6, [0, 1, 2, 3], "a")
            s_blocks(6, [4, 5, 6], "b")
            s_blocks(7, [0, 1, 2, 3], "a")
            s_blocks(7, [4, 5, 6, 7], "b")
            # c4 (slots 8, 9)
            kq_proj(8, 2)
            v_proj(8, 2)
            vz_make(0)
            vz_make(1)
            s_blocks(0, [1], "c")
            s_blocks(1, [2, 3], "c")
            # c5 (slots 10, 11)
            kq_proj(10, 2)
            s_blocks(2, [3, 4, 5], "c")
            v_proj(10, 2)
            s_blocks(3, [4, 5, 6, 7], "c")
            vz_make(2)
            vz_make(3)
            pv(0)
            pv(1)
            s_blocks(4, [5, 6, 7, 8], "c")
            s_blocks(5, [6, 7, 8, 9], "c")
            s_blocks(6, [7, 8, 9, 10], "c")
            s_blocks(7, [8, 9, 10, 11], "c")
            # c6 (slots 12, 13)
            kq_proj(12, 2)
            pv(2)
            v_proj(12, 2)
            pv(3)
            vz_make(4)
            vz_make(5)
            s_blocks(4, [9], "d")
            s_blocks(5, [10, 11], "d")
            s_blocks(6, [11, 12], "d")
            s_blocks(7, [12, 13], "d")
            pv(4)
            # c7 (slots 14, 15)
            pv(5)
            kq_proj(14, 1)
            kq_proj(15, 1)
            v_proj(14, 2)
            vz_make(6)
            vz_make(7)
            pv_mm(6, list(range(13)), True, False)     # PVa6: blocks 0..12
            pv_mm(7, list(range(14)), True, False)     # PVa7: blocks 0..13
            s_blocks(6, [13], "e")
            s_blocks(7, [14, 15], "e")
            pv_mm(6, [13], False, True)                # PVb6
            pv_mm(7, [14, 15], False, True)            # PVb7
            nc.sync.dma_start(y_d[:, 6:8, :], yout[:, 6:8, :])

    nc.compile()
    return nc


def _host_inputs(x, Wq, Wk, Wv):
    """Per-core input maps. Core c = 2*b + jj."""
    x16 = x.astype(np.float16)
    wkq = np.empty((8, 128, 128), dtype=np.float16)
    wk16 = Wk.astype(np.float16)
    wq16 = Wq.astype(np.float16)
    for ch in range(8):
        wkq[ch, :, 0:DK] = wk16[ch * 128:(ch + 1) * 128, :]
        wkq[ch, :, DK:128] = wq16[ch * 128:(ch + 1) * 128, :]
    wkq = np.ascontiguousarray(wkq.transpose(1, 0, 2))      # [128, 8, 128]
    wv_h = np.ascontiguousarray(
        (Wv / 8.0).astype(np.float16).reshape(8, 128, DK).transpose(1, 0, 2))
    tri = (np.arange(128)[:, None] <= np.arange(128)[None, :])
    in_maps = []
    for core in range(8):
        b, jj = divmod(core, 2)
        sel = [int(k >= 4) if jj == 0 else int(k < 4) for k in range(8)]
        g = [2 * k + sel[k] for k in range(8)]
        cg = [2 * k + 1 - sel[k] for k in range(8)]
        slot_order = g + cg
        arr = x16[b].reshape(16, 128, 8, 128)         # [tile, r, ch, p]
        xt = np.ascontiguousarray(
            arr[slot_order].transpose(3, 0, 2, 1).reshape(128, NSLOT, 1024))
        msk = np.zeros((128, 136), dtype=np.float32)
        msk[:, 0:128] = tri
        msk[:, 128:136] = np.asarray(sel, dtype=np.float32)
        in_maps.append({
            "xt": xt,
            "wkq": wkq,
            "wv": wv_h,
            "msk": msk.astype(ml_dtypes.bfloat16),
        })
    return in_maps


def kernel(x, Wq, Wk, Wv):
    from concourse.bass_utils import run_bass_kernel_spmd

    x = np.asarray(x, dtype=np.float32)
    Wq = np.asarray(Wq, dtype=np.float32)
    Wk = np.asarray(Wk, dtype=np.float32)
    Wv = np.asarray(Wv, dtype=np.float32)

    if "nc" not in _CACHE:
        _CACHE["nc"] = _build()
    nc = _CACHE["nc"]

    in_maps = _host_inputs(x, Wq, Wk, Wv)
    res = run_bass_kernel_spmd(nc, in_maps, core_ids=list(range(8)))
    out = np.empty((B, T, DK), dtype=np.float32)
    for core in range(8):
        b, jj = divmod(core, 2)
        sel = [int(k >= 4) if jj == 0 else int(k < 4) for k in range(8)]
        yloc = res.results[core]["y"]                 # [128, 8, 64]
        for li in range(NLI):
            gt = 2 * li + sel[li]
            out[b, gt * 128:(gt + 1) * 128, :] = yloc[:, li, :]
    return out
